# revision 1
# baseline (speedup 1.0000x reference)
"""Trainium2 Bass kernel for nn_MixedHeads (causal multi-head attention).

Reference computes, per (b, h):
  Q = x[b,:, :1024] @ Wq[h,:64,:1024].T      [T, 64]
  S = Q @ K.T * 0.125, causal mask, softmax
  O = P @ V, concat heads, pad to 2048 cols.

Sharding over 8 cores: core c -> batch b=c//2, heads h0=8*(c%2) .. h0+8.
Each core only reads its batch slice of x (8 MB) and its 8 heads' weights.

Device kernel (same SPMD program on all cores, data differs per core):
  phase 0/1 (fused): DMA x tiles [128,1024], PE-transpose to xsT [e,t] chunks,
     project with packed weights Wcat [1024, 1536] = [Q(8h*64) | K | V]:
       Q^T/K^T: out = Wchunk.T @ xsT  -> packed [128=2 heads x 64d, t]
       V:       out = xsT_tile.T @ Wv -> [t, 8h*64]  (natural PV layout)
     V stored with a ones column appended per head ([s, 65]) so the PV matmul
     also produces the softmax denominator as row 64.
  phase 2: per head, flash-style in S^T layout [s, tq]:
       S^T = K^T.T @ Q^T (per 128-s-chunk x 512-tq block, causal blocks only)
       P^T = exp(0.125*S^T + mask)   (mask only on 128-wide diagonal sub-block)
       acc[tq] += V'.T @ P^T         (PSUM accumulation over s chunks)
     acc [65, 512] -> DRAM; host divides rows 0..63 by row 64 and transposes.
"""

import sys

sys.path.insert(0, "/opt/trn_rl_repo")

import numpy as np

import concourse.bass as bass
import concourse.tile as tile
from concourse import bacc, mybir
from concourse.bass_utils import run_bass_kernel_spmd

F32 = mybir.dt.float32
F32R = mybir.dt.float32r
EXP = mybir.ActivationFunctionType.Exp


def _r(ap):
    return ap.bitcast(F32R)

B, TFULL, E, D = 4, 2048, 1024, 64
HPC = 8  # heads per core
NEG = -1.0e30
SCALE = 0.125


def build_nc(T=TFULL, reps=1):
    nq = T // 512   # tq chunks
    ns = T // 128   # s chunks
    nc = bacc.Bacc(None, target_bir_lowering=False, enable_partition_id=False)
    xbt = nc.dram_tensor("xbt", [E, T], F32, kind="ExternalInput")
    wcat = nc.dram_tensor("wcat", [E, 3 * HPC * D], F32, kind="ExternalInput")
    maskd = nc.dram_tensor("maskd", [128, 128], F32, kind="ExternalInput")
    o = nc.dram_tensor("o", [HPC, nq, 65, 512], F32, kind="ExternalOutput")

    with tile.TileContext(nc) as tc:
        with (
            tc.tile_pool(name="const", bufs=1) as constp,
            tc.tile_pool(name="qkstore", bufs=1) as qkp,
            tc.tile_pool(name="vstore", bufs=1) as vp,
        ):
            mask = constp.tile([128, 128], F32, tag="mask")
            nc.sync.dma_start(mask[:], maskd[:])
            Qs = qkp.tile([128, 4 * T], F32R, tag="qs")
            Ks = qkp.tile([128, 4 * T], F32R, tag="ks")
            Vs = vp.tile([128, ns * 520], F32R, tag="vs")
            nc.gpsimd.memset(Vs[:].bitcast(F32), 1.0)

            def emit_body():
                # ---------------- projection phase ----------------
                with (
                    tc.tile_pool(name="wpool", bufs=1) as wp,
                    tc.tile_pool(name="xsT", bufs=2) as xtp,
                    tc.tile_pool(name="prpsum", bufs=4, space="PSUM") as prp,
                ):
                    W = wp.tile([128, 8 * 1536], F32R, tag="w")
                    for e in range(8):
                        nc.sync.dma_start(
                            W[:, 1536 * e : 1536 * (e + 1)],
                            wcat[128 * e : 128 * (e + 1), :].bitcast(F32R),
                        )
                    for q in range(nq):
                        xsT = xtp.tile([128, 8 * 512], F32R, tag="xst")
                        nc.sync.dma_start(
                            xsT[:].rearrange("p (ec c) -> p ec c", ec=8),
                            xbt[:, 512 * q : 512 * (q + 1)]
                            .bitcast(F32R)
                            .rearrange("(ec p) c -> p ec c", p=128),
                        )
                        # Q^T / K^T groups: stationary = W cols, moving = xsT
                        for g in range(8):
                            pg = prp.tile([128, 512], F32, tag="pp")
                            for e in range(8):
                                nc.tensor.matmul(
                                    pg[:],
                                    W[:, 1536 * e + 128 * g : 1536 * e + 128 * (g + 1)],
                                    xsT[:, 512 * e : 512 * (e + 1)],
                                    start=(e == 0),
                                    stop=(e == 7),
                                )
                            dst = Qs if g < 4 else Ks
                            gg = g % 4
                            nc.scalar.copy(
                                dst[:, T * gg + 512 * q : T * gg + 512 * (q + 1)], pg[:]
                            )
                        # V: stationary = xsT tile, moving = W V-cols
                        for i in range(4):
                            pv = prp.tile([128, 512], F32, tag="pp")
                            for e in range(8):
                                nc.tensor.matmul(
                                    pv[:],
                                    xsT[:, 512 * e + 128 * i : 512 * e + 128 * (i + 1)],
                                    W[:, 1536 * e + 1024 : 1536 * e + 1536],
                                    start=(e == 0),
                                    stop=(e == 7),
                                )
                            c = 4 * q + i
                            nc.vector.tensor_copy(
                                Vs[:, 520 * c : 520 * c + 520].rearrange(
                                    "p (h d) -> p h d", h=8
                                )[:, :, 0:64],
                                pv[:].rearrange("p (h d) -> p h d", h=8),
                            )

                # ---------------- attention phase ----------------
                with (
                    tc.tile_pool(name="ppool", bufs=4) as ppool,
                    tc.tile_pool(name="ostage", bufs=2) as osp,
                    tc.tile_pool(name="spsum", bufs=2, space="PSUM") as spp,
                    tc.tile_pool(name="accpsum", bufs=4, space="PSUM") as accp,
                ):
                    for h in range(HPC):
                        row = 64 * (h % 2)
                        cb = T * (h // 2)
                        accs = [
                            accp.tile([128, 512], F32, tag="acc", name=f"acc{h}_{i}")
                            for i in range(nq)
                        ]
                        for j in range(ns):
                            i0 = j // 4
                            c0 = 128 * (j % 4)
                            pts = {}
                            # pair blocks two-at-a-time into [128,1024] psum
                            # tiles so one exp covers both (halves ACT count)
                            blocks = list(range(i0, nq))
                            for b0 in range(0, len(blocks), 2):
                                grp = blocks[b0 : b0 + 2]
                                w = 512 * len(grp)
                                spt = spp.tile([128, 1024], F32, tag="sp")
                                pt = ppool.tile([128, 1024], F32R, tag="pt")
                                lo = None
                                for k, i in enumerate(grp):
                                    n0 = c0 if i == i0 else 0
                                    if lo is None:
                                        lo = 512 * k + n0
                                    nc.tensor.matmul(
                                        spt[:, 512 * k + n0 : 512 * (k + 1)],
                                        Ks[
                                            row : row + 64,
                                            cb + 128 * j : cb + 128 * (j + 1),
                                        ],
                                        Qs[
                                            row : row + 64,
                                            cb + 512 * i + n0 : cb + 512 * (i + 1),
                                        ],
                                        start=True,
                                        stop=True,
                                    )
                                    pts[i] = (pt, 512 * k)
                                if grp[0] == i0:
                                    nc.vector.tensor_add(
                                        spt[:, c0 : c0 + 128],
                                        spt[:, c0 : c0 + 128],
                                        mask[:],
                                    )
                                nc.scalar.activation(
                                    pt[:, lo:w], spt[:, lo:w], EXP, scale=SCALE
                                )
                            for i in range(i0, nq):
                                pt, off = pts[i]
                                if i == i0 and c0 > 0:
                                    nc.gpsimd.memset(
                                        pt[:, off : off + c0].bitcast(F32), 0.0
                                    )
                                nc.tensor.matmul(
                                    accs[i][0:65, :],
                                    Vs[:, 520 * j + 65 * h : 520 * j + 65 * h + 65],
                                    pt[:, off : off + 512],
                                    start=(j == 0),
                                    stop=(j == 4 * i + 3),
                                )
                            if j % 4 == 3:
                                i = i0
                                ot = osp.tile([128, 512], F32, tag="ot")
                                nc.vector.tensor_copy(ot[0:65, :], accs[i][0:65, :])
                                nc.sync.dma_start(o[h, i], ot[0:65, :])

            for _rep in range(reps):
                emit_body()

    nc.compile()
    return nc


def make_in_maps(x, Wq, Wk, Wv, T=TFULL):
    x = np.asarray(x, np.float32)
    mask = np.where(
        np.arange(128)[None, :] >= np.arange(128)[:, None], 0.0, NEG
    ).astype(np.float32)
    in_maps = []
    for c in range(8):
        b, h0 = c // 2, HPC * (c % 2)
        xbv = np.ascontiguousarray(x[b, :T, :E].T)  # [E, T]
        parts = []
        for Wg in (Wq, Wk, Wv):
            wg = np.asarray(Wg, np.float32)[h0 : h0 + HPC, :D, :E]  # [8, 64, 1024]
            parts.append(wg.transpose(2, 0, 1).reshape(E, HPC * D))
        wcat = np.ascontiguousarray(np.concatenate(parts, axis=1))  # [1024, 1536]
        in_maps.append({"xbt": xbv, "wcat": wcat, "maskd": mask})
    return in_maps


def assemble(results, T=TFULL):
    out = np.zeros((B, TFULL, 2048), np.float32)
    for c in range(8):
        b, h0 = c // 2, HPC * (c % 2)
        ov = np.asarray(results[c]["o"])  # [8, nq, 65, 512]
        On = ov[:, :, :64, :] / ov[:, :, 64:65, :]  # [8, nq, 64, 512]
        blk = On.transpose(1, 3, 0, 2).reshape(T, HPC * D)  # [(i f), (h d)]
        out[b, :T, D * h0 : D * h0 + HPC * D] = blk
    return out


def kernel(**inputs):
    nc = build_nc()
    in_maps = make_in_maps(inputs["x"], inputs["Wq"], inputs["Wk"], inputs["Wv"])
    res = run_bass_kernel_spmd(nc, in_maps, core_ids=list(range(8)))
    return assemble(res.results)



# revision 3
# speedup vs baseline: 1.7816x; 1.7816x over previous
"""Trainium2 Bass kernel for nn_MixedHeads (causal MHA) -- v9.

v8 features (fp8 DoubleRow QK-proj and PV, bf16 S/V-proj, fp8 exp output,
post-exp tril masking off the ACT path, fp32 guards for t<128, pipelined PV)
PLUS: the tq-block loop is outermost and interleaved with the projection --
attention for block i=q is emitted right after projection tile q, so the ACT
engine (the end-to-end bottleneck) starts exp work ~40us earlier instead of
idling through the projection. PSUM rebudget: proj pool 2 banks, spt 2x2,
acc 2x1.
"""

import os
import sys

sys.path.insert(0, "/opt/trn_rl_repo")

import numpy as np
import ml_dtypes

import concourse.bass as bass
import concourse.tile as tile
from concourse import bacc, mybir
from concourse.bass_utils import run_bass_kernel_spmd

F32 = mybir.dt.float32
F32R = mybir.dt.float32r
BF16 = mybir.dt.bfloat16
F8 = mybir.dt.float8e4
EXP = mybir.ActivationFunctionType.Exp
DRM = mybir.MatmulPerfMode.DoubleRow

B, TFULL, E, D = 4, 2048, 1024, 64
HPC = 8  # heads per core
SCALE = 0.125
NP_F8 = ml_dtypes.float8_e4m3


def build_nc(T=TFULL, reps=1):
    nq = T // 512   # tq blocks
    ns = T // 128   # s chunks
    njj = ns // 2   # s chunk pairs
    nc = bacc.Bacc(None, target_bir_lowering=False, enable_partition_id=False)
    xbt = nc.dram_tensor("xbt", [E, T], BF16, kind="ExternalInput")
    xq8d = nc.dram_tensor("xq8", [128, 4, 2, T], F8, kind="ExternalInput")
    wvd = nc.dram_tensor("wv", [E, 512], BF16, kind="ExternalInput")
    wqk32d = nc.dram_tensor("wqk32", [E, 1024], BF16, kind="ExternalInput")
    wqk8d = nc.dram_tensor("wqk8", [128, 4, 2, 1024], F8, kind="ExternalInput")
    tril8d = nc.dram_tensor("tril8", [128, 128], F8, kind="ExternalInput")
    trilfd = nc.dram_tensor("trilf", [128, 128], F32, kind="ExternalInput")
    o = nc.dram_tensor("o", [HPC, nq, 65, 512], F32, kind="ExternalOutput")

    with tile.TileContext(nc) as tc:
        with (
            tc.tile_pool(name="const", bufs=1) as constp,
            tc.tile_pool(name="qkstore", bufs=1) as qkp,
            tc.tile_pool(name="vstore", bufs=1) as vp,
        ):
            tril8 = constp.tile([128, 128], F8, tag="tril8")
            nc.sync.dma_start(tril8[:], tril8d[:])
            trilf = constp.tile([128, 128], F32R, tag="trilf")
            nc.sync.dma_start(trilf[:], trilfd[:].bitcast(F32R))
            Qs = qkp.tile([128, 4 * T], BF16, tag="qs")
            Ks = qkp.tile([128, 4 * T], BF16, tag="ks")
            Qf = qkp.tile([128, 4 * 128], BF16, tag="qf")
            Kf = qkp.tile([128, 4 * 128], BF16, tag="kf")
            # Vs8[p, jj, half, h, 0:64]=V, col 64 = 1.0 (denominator row)
            Vs8 = vp.tile([128, njj * 2 * HPC * 80], F8, tag="vs8")
            V0f = vp.tile([128, HPC * 80], F32R, tag="v0f")
            nc.gpsimd.memset(Vs8[:], 1.0)
            nc.gpsimd.memset(V0f[:].bitcast(F32), 1.0)

            def vs8_lhsT(jj, h):
                return Vs8[:, 1280 * jj : 1280 * (jj + 1)].rearrange(
                    "p (h2 hh c) -> p h2 hh c", h2=2, hh=HPC
                )[:, :, h, 0:65]

            def emit_body():
                with (
                    tc.tile_pool(name="wpool", bufs=1) as wp,
                    tc.tile_pool(name="xsT", bufs=2) as xtp,
                    tc.tile_pool(name="x8p", bufs=2) as x8p,
                    tc.tile_pool(name="ptqp", bufs=3) as ptqp,
                    tc.tile_pool(name="pt0p", bufs=2) as pt0p,
                    tc.tile_pool(name="ostage", bufs=2) as osp,
                    tc.tile_pool(name="prpsum", bufs=2, space="PSUM") as prp,
                    tc.tile_pool(name="spsum", bufs=2, space="PSUM") as spp,
                    tc.tile_pool(name="accpsum", bufs=2, space="PSUM") as accp,
                ):
                    W8 = wp.tile([128, 4, 2, 1024], F8, tag="w8")
                    nc.sync.dma_start(W8[:], wqk8d[:])
                    Wv = wp.tile([128, 8, 512], BF16, tag="wv")
                    nc.sync.dma_start(
                        Wv[:], wvd[:].rearrange("(ec p) m -> p ec m", p=128)
                    )
                    Wqk32 = wp.tile([128, 8, 1024], BF16, tag="wqk32")
                    nc.sync.dma_start(
                        Wqk32[:], wqk32d[:].rearrange("(ec p) m -> p ec m", p=128)
                    )

                    pending_pv = [None]

                    def flush_pv():
                        if pending_pv[0] is not None:
                            pending_pv[0]()
                            pending_pv[0] = None

                    def proj_tile(q):
                        xsT = xtp.tile([128, 8, 512], BF16, tag="xst")
                        nc.sync.dma_start(
                            xsT[:],
                            xbt[:, 512 * q : 512 * (q + 1)].rearrange(
                                "(ec p) c -> p ec c", p=128
                            ),
                        )
                        x8 = x8p.tile([128, 4, 2, 512], F8, tag="x8")
                        nc.sync.dma_start(
                            x8[:], xq8d[:, :, :, 512 * q : 512 * (q + 1)]
                        )
                        if q == 0:
                            # fp32-path mini-projection of Q/K for t in [0,128)
                            for g in range(8):
                                mg = prp.tile([128, 512], F32, tag="pp")
                                for e in range(8):
                                    nc.tensor.matmul(
                                        mg[:, 0:128],
                                        Wqk32[:, e, 128 * g : 128 * (g + 1)],
                                        xsT[:, e, 0:128],
                                        start=(e == 0),
                                        stop=(e == 7),
                                    )
                                dstf = Qf if g < 4 else Kf
                                gg = g % 4
                                nc.vector.tensor_copy(
                                    dstf[:, 128 * gg : 128 * (gg + 1)], mg[:, 0:128]
                                )
                        for g in range(8):
                            pg = prp.tile([128, 512], F32, tag="pp")
                            for c in range(4):
                                nc.tensor.matmul(
                                    pg[:],
                                    W8[:, c, :, 128 * g : 128 * (g + 1)],
                                    x8[:, c, :, :],
                                    start=(c == 0),
                                    stop=(c == 3),
                                    perf_mode=DRM,
                                )
                            dst = Qs if g < 4 else Ks
                            gg = g % 4
                            nc.vector.tensor_copy(
                                dst[:, T * gg + 512 * q : T * gg + 512 * (q + 1)],
                                pg[:],
                            )
                        for iv in range(4):
                            pv = prp.tile([128, 512], F32, tag="pp")
                            for e in range(8):
                                nc.tensor.matmul(
                                    pv[:],
                                    xsT[:, e, 128 * iv : 128 * (iv + 1)],
                                    Wv[:, e, :],
                                    start=(e == 0),
                                    stop=(e == 7),
                                )
                            cchunk = 4 * q + iv
                            jj, half = cchunk // 2, cchunk % 2
                            nc.vector.tensor_copy(
                                Vs8[
                                    :,
                                    1280 * jj
                                    + 640 * half : 1280 * jj
                                    + 640 * half
                                    + 640,
                                ].rearrange("p (hh c) -> p hh c", hh=HPC)[:, :, 0:64],
                                pv[:].rearrange("p (h d) -> p h d", h=HPC),
                            )
                            if cchunk == 0:
                                nc.vector.tensor_copy(
                                    V0f[:].rearrange("p (hh c) -> p hh c", hh=HPC)[
                                        :, :, 0:64
                                    ],
                                    pv[:].rearrange("p (h d) -> p h d", h=HPC),
                                )

                    def attn_block(i):
                        for h in range(HPC):
                            row = 64 * (h % 2)
                            cb = T * (h // 2)
                            fb = 128 * (h // 2)
                            acc = accp.tile(
                                [128, 512], F32, tag="acc", name=f"acc{i}_{h}"
                            )
                            for jj in range(2 * i + 2):
                                # ptq: [half(2) x 512] fp8 P^T for (j0,j1), blk i
                                ptq = ptqp.tile([128, 1024], F8, tag="ptq")
                                spt = spp.tile([128, 1024], F32, tag="sp")
                                pt0 = None
                                n0s = []
                                for half in range(2):
                                    j = 2 * jj + half
                                    diag = j // 4 == i
                                    n0 = 128 * (j % 4) if diag else 0
                                    n0s.append(n0)
                                    hb = 512 * half
                                    if i == 0 and j == 0:
                                        nc.tensor.matmul(
                                            spt[:, 0:128],
                                            Kf[row : row + 64, fb : fb + 128],
                                            Qf[row : row + 64, fb : fb + 128],
                                            start=True,
                                            stop=True,
                                        )
                                        nc.tensor.matmul(
                                            spt[:, 128:512],
                                            Ks[row : row + 64, cb : cb + 128],
                                            Qs[row : row + 64, cb + 128 : cb + 512],
                                            start=True,
                                            stop=True,
                                        )
                                    else:
                                        nc.tensor.matmul(
                                            spt[:, hb + n0 : hb + 512],
                                            Ks[
                                                row : row + 64,
                                                cb + 128 * j : cb + 128 * (j + 1),
                                            ],
                                            Qs[
                                                row : row + 64,
                                                cb
                                                + 512 * i
                                                + n0 : cb
                                                + 512 * (i + 1),
                                            ],
                                            start=True,
                                            stop=True,
                                        )
                                diagpair = jj >= 2 * i
                                if diagpair:
                                    # trimmed halves -> one act per half
                                    for half in range(2):
                                        hb = 512 * half
                                        nc.scalar.activation(
                                            ptq[:, hb + n0s[half] : hb + 512],
                                            spt[:, hb + n0s[half] : hb + 512],
                                            EXP,
                                            scale=SCALE,
                                        )
                                        # causal tril on the 128-wide diag block
                                        dg = hb + n0s[half]
                                        nc.gpsimd.tensor_mul(
                                            ptq[:, dg : dg + 128],
                                            ptq[:, dg : dg + 128],
                                            tril8[:],
                                        )
                                else:
                                    nc.scalar.activation(
                                        ptq[:, 0:1024],
                                        spt[:, 0:1024],
                                        EXP,
                                        scale=SCALE,
                                    )
                                if i == 0 and jj == 0:
                                    pt0 = pt0p.tile([128, 128], F32R, tag="pt0")
                                    nc.scalar.activation(
                                        pt0[:], spt[:, 0:128], EXP, scale=SCALE
                                    )
                                    nc.vector.tensor_mul(pt0[:], pt0[:], trilf[:])
                                # zero the unwritten lead-in of the j1 half that
                                # the PV read range covers
                                if jj == 2 * i and i > 0:
                                    nc.gpsimd.memset(ptq[:, 512:640], 0.0)
                                if jj == 2 * i + 1:
                                    nc.gpsimd.memset(ptq[:, 768:896], 0.0)

                                flush_pv()

                                def make_pv(
                                    h=h, i=i, jj=jj, ptq=ptq, pt0=pt0, acc=acc
                                ):
                                    def emit_pv():
                                        ptq2 = ptq[:].rearrange(
                                            "p (h2 c) -> p h2 c", h2=2
                                        )
                                        if jj == 2 * i + 1:
                                            cc0 = 256
                                        elif jj == 2 * i and i == 0:
                                            cc0 = 128
                                        else:
                                            cc0 = 0
                                        nc.tensor.matmul(
                                            acc[0:65, cc0:512],
                                            vs8_lhsT(jj, h),
                                            ptq2[:, :, cc0:512],
                                            start=(jj == 0),
                                            stop=(jj == 2 * i + 1),
                                            perf_mode=DRM,
                                        )
                                        if i == 0 and jj == 0:
                                            nc.tensor.matmul(
                                                acc[0:65, 0:128],
                                                V0f[:, 80 * h : 80 * h + 65],
                                                pt0[:],
                                                start=False,
                                                stop=False,
                                            )
                                        if jj == 2 * i + 1:
                                            ot = osp.tile([128, 512], F32, tag="ot")
                                            nc.vector.tensor_copy(
                                                ot[0:65, :], acc[0:65, :]
                                            )
                                            nc.sync.dma_start(o[h, i], ot[0:65, :])

                                    return emit_pv

                                pending_pv[0] = make_pv()

                    for q in range(nq):
                        proj_tile(q)
                        attn_block(q)
                    flush_pv()

            for _rep in range(reps):
                emit_body()

    nc.compile()
    return nc


def make_in_maps(x, Wq, Wk, Wv, T=TFULL):
    x = np.asarray(x, np.float32)
    trilf = (np.arange(128)[None, :] >= np.arange(128)[:, None]).astype(np.float32)
    in_maps = []
    for c in range(8):
        b, h0 = c // 2, HPC * (c % 2)
        xbv = np.ascontiguousarray(x[b, :T, :E].T)  # [E, T]
        xq8 = np.ascontiguousarray(
            xbv.reshape(4, 2, 128, T).transpose(2, 0, 1, 3)
        ).astype(NP_F8)
        parts = []
        for Wg in (Wq, Wk, Wv):
            wg = np.asarray(Wg, np.float32)[h0 : h0 + HPC, :D, :E]  # [8, 64, 1024]
            parts.append(wg.transpose(2, 0, 1).reshape(E, HPC * D))
        wqk = np.concatenate(parts[:2], axis=1)  # [E, 1024]
        wqk8 = np.ascontiguousarray(
            wqk.reshape(4, 2, 128, 1024).transpose(2, 0, 1, 3)
        ).astype(NP_F8)
        wv = np.ascontiguousarray(parts[2])  # [E, 512]
        in_maps.append(
            {
                "xbt": xbv.astype(ml_dtypes.bfloat16),
                "xq8": xq8,
                "wv": wv.astype(ml_dtypes.bfloat16),
                "wqk32": np.ascontiguousarray(wqk).astype(ml_dtypes.bfloat16),
                "wqk8": wqk8,
                "tril8": trilf.astype(NP_F8),
                "trilf": trilf,
            }
        )
    return in_maps


def assemble(results, T=TFULL):
    out = np.zeros((B, TFULL, 2048), np.float32)
    for c in range(8):
        b, h0 = c // 2, HPC * (c % 2)
        ov = np.asarray(results[c]["o"])  # [8, nq, 65, 512]
        On = ov[:, :, :64, :] / ov[:, :, 64:65, :]  # [8, nq, 64, 512]
        blk = On.transpose(1, 3, 0, 2).reshape(T, HPC * D)  # [(i f), (h d)]
        out[b, :T, D * h0 : D * h0 + HPC * D] = blk
    return out


def kernel(**inputs):
    nc = build_nc()
    in_maps = make_in_maps(inputs["x"], inputs["Wq"], inputs["Wk"], inputs["Wv"])
    res = run_bass_kernel_spmd(nc, in_maps, core_ids=list(range(8)))
    return assemble(res.results)


# revision 4
# speedup vs baseline: 1.8197x; 1.0214x over previous
"""Trainium2 Bass kernel for nn_MixedHeads (causal MHA) -- v9.

v8 features (fp8 DoubleRow QK-proj and PV, bf16 S/V-proj, fp8 exp output,
post-exp tril masking off the ACT path, fp32 guards for t<128, pipelined PV)
PLUS: the tq-block loop is outermost and interleaved with the projection --
attention for block i=q is emitted right after projection tile q, so the ACT
engine (the end-to-end bottleneck) starts exp work ~40us earlier instead of
idling through the projection. PSUM rebudget: proj pool 2 banks, spt 2x2,
acc 2x1.
"""

import os
import sys

sys.path.insert(0, "/opt/trn_rl_repo")

import numpy as np
import ml_dtypes

import concourse.bass as bass
import concourse.tile as tile
from concourse import bacc, mybir
from concourse.bass_utils import run_bass_kernel_spmd

F32 = mybir.dt.float32
F32R = mybir.dt.float32r
BF16 = mybir.dt.bfloat16
F8 = mybir.dt.float8e4
EXP = mybir.ActivationFunctionType.Exp
DRM = mybir.MatmulPerfMode.DoubleRow

B, TFULL, E, D = 4, 2048, 1024, 64
HPC = 8  # heads per core
SCALE = 0.125
NP_F8 = ml_dtypes.float8_e4m3


def build_nc(T=TFULL, reps=1):
    nq = T // 512   # tq blocks
    ns = T // 128   # s chunks
    njj = ns // 2   # s chunk pairs
    nc = bacc.Bacc(None, target_bir_lowering=False, enable_partition_id=False)
    xbt = nc.dram_tensor("xbt", [E, T], BF16, kind="ExternalInput")
    xq8d = nc.dram_tensor("xq8", [128, 4, 2, T], F8, kind="ExternalInput")
    wvd = nc.dram_tensor("wv", [E, 512], BF16, kind="ExternalInput")
    wqk32d = nc.dram_tensor("wqk32", [E, 1024], BF16, kind="ExternalInput")
    wqk8d = nc.dram_tensor("wqk8", [128, 4, 2, 1024], F8, kind="ExternalInput")
    tril8d = nc.dram_tensor("tril8", [128, 128], F8, kind="ExternalInput")
    trilfd = nc.dram_tensor("trilf", [128, 128], F32, kind="ExternalInput")
    o = nc.dram_tensor("o", [HPC, nq, 65, 512], F32, kind="ExternalOutput")

    with tile.TileContext(nc) as tc:
        with (
            tc.tile_pool(name="const", bufs=1) as constp,
            tc.tile_pool(name="qkstore", bufs=1) as qkp,
            tc.tile_pool(name="vstore", bufs=1) as vp,
        ):
            tril8 = constp.tile([128, 128], F8, tag="tril8")
            nc.sync.dma_start(tril8[:], tril8d[:])
            trilf = constp.tile([128, 128], F32R, tag="trilf")
            nc.sync.dma_start(trilf[:], trilfd[:].bitcast(F32R))
            Qs = qkp.tile([128, 4 * T], BF16, tag="qs")
            Ks = qkp.tile([128, 4 * T], BF16, tag="ks")
            Qf = qkp.tile([128, 4 * 128], BF16, tag="qf")
            Kf = qkp.tile([128, 4 * 128], BF16, tag="kf")
            # Vs8[p, jj, half, h, 0:64]=V, col 64 = 1.0 (denominator row)
            Vs8 = vp.tile([128, njj * 2 * HPC * 80], F8, tag="vs8")
            V0f = vp.tile([128, HPC * 80], F32R, tag="v0f")
            nc.gpsimd.memset(Vs8[:], 1.0)
            nc.gpsimd.memset(V0f[:].bitcast(F32), 1.0)

            def vs8_lhsT(jj, h):
                return Vs8[:, 1280 * jj : 1280 * (jj + 1)].rearrange(
                    "p (h2 hh c) -> p h2 hh c", h2=2, hh=HPC
                )[:, :, h, 0:65]

            def emit_body():
                with (
                    tc.tile_pool(name="wpool", bufs=1) as wp,
                    tc.tile_pool(name="xsT", bufs=2) as xtp,
                    tc.tile_pool(name="x8p", bufs=2) as x8p,
                    tc.tile_pool(name="ptqp", bufs=4) as ptqp,
                    tc.tile_pool(name="pt0p", bufs=2) as pt0p,
                    tc.tile_pool(name="ostage", bufs=2) as osp,
                    tc.tile_pool(name="prpsum", bufs=1, space="PSUM") as prp,
                    tc.tile_pool(name="spsum", bufs=3, space="PSUM") as spp,
                    tc.tile_pool(name="accpsum", bufs=1, space="PSUM") as accp,
                ):
                    W8 = wp.tile([128, 4, 2, 1024], F8, tag="w8")
                    nc.sync.dma_start(W8[:], wqk8d[:])
                    Wv = wp.tile([128, 8, 512], BF16, tag="wv")
                    nc.sync.dma_start(
                        Wv[:], wvd[:].rearrange("(ec p) m -> p ec m", p=128)
                    )
                    Wqk32 = wp.tile([128, 8, 1024], BF16, tag="wqk32")
                    nc.sync.dma_start(
                        Wqk32[:], wqk32d[:].rearrange("(ec p) m -> p ec m", p=128)
                    )

                    pending_pv = []

                    def flush_pv(limit=0):
                        while len(pending_pv) > limit:
                            pending_pv.pop(0)()

                    def proj_tile(q):
                        xsT = xtp.tile([128, 8, 512], BF16, tag="xst")
                        nc.sync.dma_start(
                            xsT[:],
                            xbt[:, 512 * q : 512 * (q + 1)].rearrange(
                                "(ec p) c -> p ec c", p=128
                            ),
                        )
                        x8 = x8p.tile([128, 4, 2, 512], F8, tag="x8")
                        nc.sync.dma_start(
                            x8[:], xq8d[:, :, :, 512 * q : 512 * (q + 1)]
                        )
                        if q == 0:
                            # fp32-path mini-projection of Q/K for t in [0,128)
                            for g in range(8):
                                mg = prp.tile([128, 512], F32, tag="pp")
                                for e in range(8):
                                    nc.tensor.matmul(
                                        mg[:, 0:128],
                                        Wqk32[:, e, 128 * g : 128 * (g + 1)],
                                        xsT[:, e, 0:128],
                                        start=(e == 0),
                                        stop=(e == 7),
                                    )
                                dstf = Qf if g < 4 else Kf
                                gg = g % 4
                                nc.vector.tensor_copy(
                                    dstf[:, 128 * gg : 128 * (gg + 1)], mg[:, 0:128]
                                )
                        for g in range(8):
                            pg = prp.tile([128, 512], F32, tag="pp")
                            for c in range(4):
                                nc.tensor.matmul(
                                    pg[:],
                                    W8[:, c, :, 128 * g : 128 * (g + 1)],
                                    x8[:, c, :, :],
                                    start=(c == 0),
                                    stop=(c == 3),
                                    perf_mode=DRM,
                                )
                            dst = Qs if g < 4 else Ks
                            gg = g % 4
                            nc.vector.tensor_copy(
                                dst[:, T * gg + 512 * q : T * gg + 512 * (q + 1)],
                                pg[:],
                            )
                        for iv in range(4):
                            pv = prp.tile([128, 512], F32, tag="pp")
                            for e in range(8):
                                nc.tensor.matmul(
                                    pv[:],
                                    xsT[:, e, 128 * iv : 128 * (iv + 1)],
                                    Wv[:, e, :],
                                    start=(e == 0),
                                    stop=(e == 7),
                                )
                            cchunk = 4 * q + iv
                            jj, half = cchunk // 2, cchunk % 2
                            nc.vector.tensor_copy(
                                Vs8[
                                    :,
                                    1280 * jj
                                    + 640 * half : 1280 * jj
                                    + 640 * half
                                    + 640,
                                ].rearrange("p (hh c) -> p hh c", hh=HPC)[:, :, 0:64],
                                pv[:].rearrange("p (h d) -> p h d", h=HPC),
                            )
                            if cchunk == 0:
                                nc.vector.tensor_copy(
                                    V0f[:].rearrange("p (hh c) -> p hh c", hh=HPC)[
                                        :, :, 0:64
                                    ],
                                    pv[:].rearrange("p (h d) -> p h d", h=HPC),
                                )

                    def attn_block(i):
                        for h in range(HPC):
                            row = 64 * (h % 2)
                            cb = T * (h // 2)
                            fb = 128 * (h // 2)
                            acc = accp.tile(
                                [128, 512], F32, tag="acc", name=f"acc{i}_{h}"
                            )
                            for jj in range(2 * i + 2):
                                # ptq: [half(2) x 512] fp8 P^T for (j0,j1), blk i
                                ptq = ptqp.tile([128, 1024], F8, tag="ptq")
                                spt = spp.tile([128, 1024], F32, tag="sp")
                                pt0 = None
                                n0s = []
                                for half in range(2):
                                    j = 2 * jj + half
                                    diag = j // 4 == i
                                    n0 = 128 * (j % 4) if diag else 0
                                    n0s.append(n0)
                                    hb = 512 * half
                                    if i == 0 and j == 0:
                                        nc.tensor.matmul(
                                            spt[:, 0:128],
                                            Kf[row : row + 64, fb : fb + 128],
                                            Qf[row : row + 64, fb : fb + 128],
                                            start=True,
                                            stop=True,
                                        )
                                        nc.tensor.matmul(
                                            spt[:, 128:512],
                                            Ks[row : row + 64, cb : cb + 128],
                                            Qs[row : row + 64, cb + 128 : cb + 512],
                                            start=True,
                                            stop=True,
                                        )
                                    else:
                                        nc.tensor.matmul(
                                            spt[:, hb + n0 : hb + 512],
                                            Ks[
                                                row : row + 64,
                                                cb + 128 * j : cb + 128 * (j + 1),
                                            ],
                                            Qs[
                                                row : row + 64,
                                                cb
                                                + 512 * i
                                                + n0 : cb
                                                + 512 * (i + 1),
                                            ],
                                            start=True,
                                            stop=True,
                                        )
                                diagpair = jj >= 2 * i
                                if diagpair:
                                    # trimmed halves -> one act per half
                                    for half in range(2):
                                        hb = 512 * half
                                        nc.scalar.activation(
                                            ptq[:, hb + n0s[half] : hb + 512],
                                            spt[:, hb + n0s[half] : hb + 512],
                                            EXP,
                                            scale=SCALE,
                                        )
                                        # causal tril on the 128-wide diag block
                                        dg = hb + n0s[half]
                                        nc.gpsimd.tensor_mul(
                                            ptq[:, dg : dg + 128],
                                            ptq[:, dg : dg + 128],
                                            tril8[:],
                                        )
                                else:
                                    nc.scalar.activation(
                                        ptq[:, 0:1024],
                                        spt[:, 0:1024],
                                        EXP,
                                        scale=SCALE,
                                    )
                                if i == 0 and jj == 0:
                                    pt0 = pt0p.tile([128, 128], F32R, tag="pt0")
                                    nc.scalar.activation(
                                        pt0[:], spt[:, 0:128], EXP, scale=SCALE
                                    )
                                    nc.vector.tensor_mul(pt0[:], pt0[:], trilf[:])
                                # zero the unwritten lead-in of the j1 half that
                                # the PV read range covers
                                if jj == 2 * i and i > 0:
                                    nc.gpsimd.memset(ptq[:, 512:640], 0.0)
                                if jj == 2 * i + 1:
                                    nc.gpsimd.memset(ptq[:, 768:896], 0.0)

                                flush_pv(limit=1)

                                def make_pv(
                                    h=h, i=i, jj=jj, ptq=ptq, pt0=pt0, acc=acc
                                ):
                                    def emit_pv():
                                        ptq2 = ptq[:].rearrange(
                                            "p (h2 c) -> p h2 c", h2=2
                                        )
                                        if jj == 2 * i + 1:
                                            cc0 = 256
                                        elif jj == 2 * i and i == 0:
                                            cc0 = 128
                                        else:
                                            cc0 = 0
                                        nc.tensor.matmul(
                                            acc[0:65, cc0:512],
                                            vs8_lhsT(jj, h),
                                            ptq2[:, :, cc0:512],
                                            start=(jj == 0),
                                            stop=(jj == 2 * i + 1),
                                            perf_mode=DRM,
                                        )
                                        if i == 0 and jj == 0:
                                            nc.tensor.matmul(
                                                acc[0:65, 0:128],
                                                V0f[:, 80 * h : 80 * h + 65],
                                                pt0[:],
                                                start=False,
                                                stop=False,
                                            )
                                        if jj == 2 * i + 1:
                                            ot = osp.tile([128, 512], F32, tag="ot")
                                            nc.vector.tensor_copy(
                                                ot[0:65, :], acc[0:65, :]
                                            )
                                            nc.sync.dma_start(o[h, i], ot[0:65, :])

                                    return emit_pv

                                pending_pv.append(make_pv())

                    for q in range(nq):
                        proj_tile(q)
                        attn_block(q)
                    flush_pv()

            for _rep in range(reps):
                emit_body()

    nc.compile()
    return nc


def make_in_maps(x, Wq, Wk, Wv, T=TFULL):
    x = np.asarray(x, np.float32)
    trilf = (np.arange(128)[None, :] >= np.arange(128)[:, None]).astype(np.float32)
    in_maps = []
    for c in range(8):
        b, h0 = c // 2, HPC * (c % 2)
        xbv = np.ascontiguousarray(x[b, :T, :E].T)  # [E, T]
        xq8 = np.ascontiguousarray(
            xbv.reshape(4, 2, 128, T).transpose(2, 0, 1, 3)
        ).astype(NP_F8)
        parts = []
        for Wg in (Wq, Wk, Wv):
            wg = np.asarray(Wg, np.float32)[h0 : h0 + HPC, :D, :E]  # [8, 64, 1024]
            parts.append(wg.transpose(2, 0, 1).reshape(E, HPC * D))
        wqk = np.concatenate(parts[:2], axis=1)  # [E, 1024]
        wqk8 = np.ascontiguousarray(
            wqk.reshape(4, 2, 128, 1024).transpose(2, 0, 1, 3)
        ).astype(NP_F8)
        wv = np.ascontiguousarray(parts[2])  # [E, 512]
        in_maps.append(
            {
                "xbt": xbv.astype(ml_dtypes.bfloat16),
                "xq8": xq8,
                "wv": wv.astype(ml_dtypes.bfloat16),
                "wqk32": np.ascontiguousarray(wqk).astype(ml_dtypes.bfloat16),
                "wqk8": wqk8,
                "tril8": trilf.astype(NP_F8),
                "trilf": trilf,
            }
        )
    return in_maps


def assemble(results, T=TFULL):
    out = np.zeros((B, TFULL, 2048), np.float32)
    for c in range(8):
        b, h0 = c // 2, HPC * (c % 2)
        ov = np.asarray(results[c]["o"])  # [8, nq, 65, 512]
        On = ov[:, :, :64, :] / ov[:, :, 64:65, :]  # [8, nq, 64, 512]
        blk = On.transpose(1, 3, 0, 2).reshape(T, HPC * D)  # [(i f), (h d)]
        out[b, :T, D * h0 : D * h0 + HPC * D] = blk
    return out


def kernel(**inputs):
    nc = build_nc()
    in_maps = make_in_maps(inputs["x"], inputs["Wq"], inputs["Wk"], inputs["Wv"])
    res = run_bass_kernel_spmd(nc, in_maps, core_ids=list(range(8)))
    return assemble(res.results)
